# revision 13
# baseline (speedup 1.0000x reference)
"""Approximate EMD loss (B=16, N=M=2048, D=3) on 8 TRN2 NeuronCores. V2.

Data-parallel over batch: each core owns 2 batch items, processed
sequentially. m-layout: partition dim = m (16 tiles of 128), free dim = n.

Key redesign vs V1 (baseline 11.8ms):
- zz[n] = sum_m w*E*d2 decomposed via d2 = xn[n] + ym[m] - 2 sum_d x_d y_d,
  so the loss colsum comes out of the SAME PE pass as s (5-wide weight
  bundle: w2, w2*ym, w2*y0..2). No V/Z tiles, no GpSimd big pass, no
  DVE/GpSimd SBUF-port contention.
- ss[m] = satr * sum_n E*a[n]: DVE tensor_tensor (2x bf16) +
  tensor_scalar accum (4x) instead of scalar_tensor_tensor (1x only).
- rd colsum chunk-major (4 x 512 cols) so the rd->a->a_b dance pipelines
  per chunk and the ss pass starts ~9us into the level instead of ~22us.
- Full-width [128,16] m/n-domain ops once per level (V1: 8 groups of
  [128,2] ops -> ~60 DVE instrs/level of pure overhead + sem stalls).
- Bundle pass is m-tile-major so E buffers free progressively; E pool of
  12 x [128,4096] double-buffers ACROSS levels within 96KB/partition.
- Setup cross matmul in fp16 (1 cyc/row vs 4 for fp32).
"""
import sys
import os
import numpy as np

sys.path.insert(0, "/opt/trn_rl_repo")

B, N, M, D = 16, 2048, 2048, 3
P = 128
NT = M // P          # 16 m-tiles
NCCH = N // 128      # 16 cols in n-domain [128,16] layout
NCORES = 8
BPC = B // NCORES    # 2 batch items per core
LEVELS = np.arange(8, -3, -0.25).astype(np.float32)
LEVELS[-1] = 0.0
NSTEPS = len(LEVELS)  # 44
NEG_START = 33        # first step with negative level (d2s re-shift point)

CH = 512              # rd/ss chunk width
NCH = N // CH         # 4 chunks
E_BUFS = int(os.environ.get("EMD_E_BUFS", "12"))
BND_CH = int(os.environ.get("EMD_BND_CH", "512"))
KERNEL_NSTEPS = int(os.environ.get("EMD_NSTEPS", str(NSTEPS)))

_CACHE = {}


def _build(nsteps):
    import concourse.bacc as bacc
    import concourse.mybir as mybir
    import concourse.bass_isa as bass_isa
    from concourse.tile import TileContext
    from concourse.alu_op_type import AluOpType

    dtf = mybir.dt.float32
    dtb = mybir.dt.bfloat16
    dth = mybir.dt.float16
    AF = mybir.ActivationFunctionType
    MUL = AluOpType.mult
    ADD = AluOpType.add

    nc = bacc.Bacc(None, target_bir_lowering=False)
    xt_d = nc.dram_tensor("xt", [BPC, D, N], dth, kind="ExternalInput")     # (-2x).T
    yt_d = nc.dram_tensor("yt", [BPC, D, M], dth, kind="ExternalInput")     # y.T
    ynt_d = nc.dram_tensor("ynt", [BPC, P, NT], dtf, kind="ExternalInput")  # ym m-major
    yc_d = nc.dram_tensor("yc", [BPC, P, 3 * NT], dtf, kind="ExternalInput")   # y coords m-major
    xadj_d = nc.dram_tensor("xadj", [BPC, 1, N], dth, kind="ExternalInput")  # xn - rowmax
    rdif_d = nc.dram_tensor("rdif", [BPC, 1, N], dth, kind="ExternalInput")  # rowmax - rowmin
    xn_nt_d = nc.dram_tensor("xn_nt", [BPC, P, NCCH], dtf, kind="ExternalInput")  # xn n-major
    xc_nt_d = nc.dram_tensor("xc_nt", [BPC, P, 3 * NCCH], dtf, kind="ExternalInput")  # x n-major
    out_d = nc.dram_tensor("out", [1, BPC], dtf, kind="ExternalOutput")

    with TileContext(nc) as tc:
        with tc.tile_pool(name="d2", bufs=1) as d2p, \
             tc.tile_pool(name="state", bufs=1) as stp:
            for b in range(BPC):
                d2_sb = d2p.tile([P, NT * N], dth, tag="d2")
                satl = stp.tile([P, NCCH], dtf, tag="satl")       # n-domain
                satr_t = stp.tile([P, NT], dtf, tag="satr")       # m-domain
                lossacc = stp.tile([P, NCCH], dtf, tag="lossacc")
                ynt_sb = stp.tile([P, NT], dtf, tag="ynt")
                yc_sb = stp.tile([P, 3 * NT], dtf, tag="yc")
                xn_nt = stp.tile([P, NCCH], dtf, tag="xn_nt")
                xc_nt = stp.tile([P, 3 * NCCH], dtf, tag="xc_nt")
                satl_row = stp.tile([1, N], dtf, tag="satl_row")
                nc.vector.memset(satl[:], 1.0)
                nc.vector.memset(satl_row[:], 1.0)
                nc.vector.memset(satr_t[:], 1.0)
                nc.vector.memset(lossacc[:], 0.0)
                nc.sync.dma_start(out=ynt_sb[:], in_=ynt_d[b])
                nc.sync.dma_start(out=yc_sb[:], in_=yc_d[b])
                nc.sync.dma_start(out=xn_nt[:], in_=xn_nt_d[b])
                nc.sync.dma_start(out=xc_nt[:], in_=xc_nt_d[b])

                # ---- setup: d2s = ((-2 y.x) + ym) + (xn - rowmax), fp16
                with tc.tile_pool(name="su", bufs=1) as su, \
                     tc.tile_pool(name="sups", bufs=4, space="PSUM") as sups, \
                     tc.tile_pool(name="sutmp", bufs=4) as sutmp:
                    xt_sb = su.tile([D, N], dth, tag="xt")
                    yt_sb = su.tile([D, M], dth, tag="yt")
                    xadj_sb = su.tile([1, N], dth, tag="xadj")
                    xadj_b = su.tile([P, N], dth, tag="xadj_b")
                    nc.sync.dma_start(out=xt_sb[:], in_=xt_d[b])
                    nc.sync.dma_start(out=yt_sb[:], in_=yt_d[b])
                    nc.sync.dma_start(out=xadj_sb[:], in_=xadj_d[b])
                    nc.gpsimd.partition_broadcast(xadj_b[:], xadj_sb[0:1, :])
                    for i in range(NT):
                        for c in range(NCH):
                            cps = sups.tile([P, CH], dtf, tag="cross")
                            nc.tensor.matmul(
                                cps[:], yt_sb[:, i * P:(i + 1) * P],
                                xt_sb[:, c * CH:(c + 1) * CH])
                            # cross+ym on ScalarE (per-partition bias), then
                            # +xadj on DVE at fp16 2x
                            ctmp = sutmp.tile([P, CH], dth, tag="ctmp")
                            nc.scalar.activation(ctmp[:], cps[:], AF.Identity,
                                                 bias=ynt_sb[:, i:i + 1])
                            nc.vector.tensor_tensor(
                                d2_sb[:, i * N + c * CH: i * N + (c + 1) * CH],
                                ctmp[:], xadj_b[:, c * CH:(c + 1) * CH], ADD)

                # ---- 44-level matching loop
                with tc.tile_pool(name="ep", bufs=E_BUFS) as ep, \
                     tc.tile_pool(name="ps", bufs=2) as psc, \
                     tc.tile_pool(name="ab", bufs=2) as abp, \
                     tc.tile_pool(name="sh", bufs=1) as shp, \
                     tc.tile_pool(name="sm", bufs=2) as sm, \
                     tc.tile_pool(name="rows", bufs=4) as rowp, \
                     tc.tile_pool(name="brows", bufs=2) as browp, \
                     tc.tile_pool(name="dscr", bufs=2, space="DRAM") as dscr, \
                     tc.tile_pool(name="rps", bufs=1, space="PSUM") as rps, \
                     tc.tile_pool(name="bps", bufs=1, space="PSUM") as bps:
                    satr_lp_prev = sm.tile([P, NT], dtb, tag="satr_lp")
                    satrp_prev = sm.tile([P, NT], dtf, tag="satrp")
                    nc.vector.memset(satr_lp_prev[:], 1.0)
                    nc.vector.memset(satrp_prev[:], 1.0)
                    for s in range(nsteps):
                        lv = float(LEVELS[s])
                        if s == NEG_START:
                            # re-shift d2s to d2 - rowmin for negative levels
                            shrow = shp.tile([1, N], dth, tag="shrow")
                            shift_b = shp.tile([P, N], dth, tag="shift_b")
                            nc.sync.dma_start(out=shrow[:], in_=rdif_d[b])
                            nc.gpsimd.partition_broadcast(shift_b[:], shrow[0:1, :])
                            for i in range(NT):
                                nc.vector.tensor_tensor(
                                    d2_sb[:, i * N:(i + 1) * N],
                                    d2_sb[:, i * N:(i + 1) * N], shift_b[:], ADD)
                        # E tiles (bf16): 8 ACTs of FD=4096, each covering 2
                        # m-tiles. E_ap(i,...) = AP for m-tile i's columns.
                        E_bufs = []
                        for j in range(NT // 2):
                            E = ep.tile([P, 2 * N], dtb, tag="E")
                            nc.scalar.activation(E[:], d2_sb[:, j * 2 * N:(j + 1) * 2 * N],
                                                 AF.Exp, scale=lv)
                            E_bufs.append(E)

                        def E_ap(i, c0, c1):
                            return E_bufs[i // 2][:, (i % 2) * N + c0:(i % 2) * N + c1]

                        # rd pass, chunk-major + pipelined a-dance + ss chunks
                        rd_ps = rps.tile([1, N], dtf, tag="rd_ps")
                        a_t = sm.tile([P, NCCH], dtb, tag="a_t")
                        a_b = abp.tile([P, N], dtb, tag="a_b")
                        scrB = dscr.tile([1, N], dtb, tag="scrB")
                        ssacc = sm.tile([P, 2 * NT], dtf, tag="ssacc")
                        sdum = psc.tile([P, 1], dtb, tag="sdum")
                        for c in range(NCH):
                            lo, hi = c * CH, (c + 1) * CH
                            for i in range(NT):
                                nc.tensor.matmul(
                                    rd_ps[0:1, lo:hi],
                                    satr_lp_prev[:, i:i + 1],
                                    E_ap(i, lo, hi),
                                    start=(i == 0), stop=(i == NT - 1))
                            # a-route chunk c, all in row form (no DMA on the
                            # critical path): a = satl/rd
                            rrowi = rowp.tile([1, CH], dtf, tag="rrowi")
                            nc.vector.reciprocal_approx_fast(
                                out=rrowi[:], in_=rd_ps[0:1, lo:hi])
                            arow = rowp.tile([1, CH], dtb, tag="arow")
                            nc.vector.tensor_tensor(arow[:], rrowi[:],
                                                    satl_row[0:1, lo:hi], MUL)
                            nc.gpsimd.partition_broadcast(a_b[:, lo:hi], arow[0:1, :])
                            # off-critical: a to n-domain for the level tail
                            nc.sync.dma_start(out=scrB[0:1, lo:hi], in_=arow[:])
                            pl, ph = 32 * c, 32 * c + 32
                            nc.sync.dma_start(
                                out=a_t[pl:ph, :],
                                in_=scrB[0:1, lo:hi].rearrange(
                                    "o (p c) -> p (o c)", p=32))
                            # ss partials per half: one fused 1x DVE op per
                            # tile: accum = sum_n (E*satr)*a_b (no writes)
                            if c % 2 == 1:
                                h = c // 2
                                hlo, hhi = h * 2 * CH, (h + 1) * 2 * CH
                                for i in range(NT):
                                    nc.vector.affine_mul_reduce(
                                        out=sdum.broadcast_to((P, 2 * CH)),
                                        accum_out=ssacc[:, h * NT + i:h * NT + i + 1],
                                        in0=E_ap(i, hlo, hhi),
                                        in1=a_b[:, hlo:hhi],
                                        scale=satrp_prev[:, i:i + 1],
                                        bias=0.0)
                        # ss (satr already folded via scale)
                        ss = sm.tile([P, NT], dtf, tag="ss")
                        nc.vector.tensor_tensor(ss[:], ssacc[:, 0:NT],
                                                ssacc[:, NT:2 * NT], ADD)
                        # m-domain: sc2, satr update, bundle weights
                        ssp = sm.tile([P, NT], dtf, tag="ssp")
                        sinv = sm.tile([P, NT], dtf, tag="sinv")
                        sc2 = sm.tile([P, NT], dtf, tag="sc2")
                        mn = sm.tile([P, NT], dtf, tag="mn")
                        satrp = sm.tile([P, NT], dtf, tag="satrp")
                        satr_lp = sm.tile([P, NT], dtb, tag="satr_lp")
                        w2 = sm.tile([P, NT], dtf, tag="w2")
                        w2all = sm.tile([P, NT * 5], dtb, tag="w2all")
                        nc.vector.tensor_scalar(out=ssp[:], in0=ss[:], scalar1=1e-9,
                                                scalar2=None, op0=ADD)
                        nc.vector.reciprocal(sinv[:], ssp[:])
                        nc.vector.tensor_tensor(sc2[:], sinv[:], satr_t[:], MUL)
                        nc.vector.tensor_scalar(out=sc2[:], in0=sc2[:], scalar1=1.0,
                                                scalar2=None, op0=AluOpType.min)
                        nc.vector.tensor_tensor(w2[:], sc2[:], satrp_prev[:], MUL)
                        # bundle lhsT, interleaved [P, NT, 5]
                        w2v = w2all[:].rearrange("p (t f) -> p t f", f=5)
                        nc.vector.tensor_copy(w2v[:, :, 0], w2[:])
                        nc.vector.tensor_tensor(w2v[:, :, 1], w2[:], ynt_sb[:], MUL)
                        nc.vector.tensor_tensor(w2v[:, :, 2], w2[:], yc_sb[:, 0:NT], MUL)
                        nc.vector.tensor_tensor(w2v[:, :, 3], w2[:], yc_sb[:, NT:2 * NT], MUL)
                        nc.vector.tensor_tensor(w2v[:, :, 4], w2[:], yc_sb[:, 2 * NT:3 * NT], MUL)
                        # satr' = satr - min(ss, satr)  (>= 0 by construction)
                        nc.vector.tensor_tensor(mn[:], ss[:], satr_t[:], AluOpType.min)
                        nc.vector.tensor_sub(satr_t[:], satr_t[:], mn[:])
                        nc.vector.tensor_scalar(out=satrp[:], in0=satr_t[:],
                                                scalar1=1e-30, scalar2=None, op0=ADD)
                        nc.vector.tensor_copy(satr_lp[:], satrp[:])
                        # bundle pass: 5-wide colsums of E, m-tile-major so E
                        # buffers free progressively for next level's ACTs
                        bnd_ps = bps.tile([5, N], dtf, tag="bnd_ps")
                        for i in range(NT):
                            for c in range(N // BND_CH):
                                lo, hi = c * BND_CH, (c + 1) * BND_CH
                                nc.tensor.matmul(
                                    bnd_ps[0:5, lo:hi],
                                    w2all[:, i * 5:(i + 1) * 5],
                                    E_ap(i, lo, hi),
                                    start=(i == 0), stop=(i == NT - 1))
                        # end dance: bundle rows -> n-domain (chunked copies)
                        scrC = dscr.tile([5, N], dtf, tag="scrC")
                        for c in range(NCH):
                            lo, hi = c * CH, (c + 1) * CH
                            brow = browp.tile([5, CH], dtf, tag="brow")
                            nc.scalar.copy(brow[:], bnd_ps[0:5, lo:hi])
                            nc.sync.dma_start(out=scrC[0:5, lo:hi], in_=brow[:])
                        bnd_nt = sm.tile([P, 5 * NCCH], dtf, tag="bnd_nt")
                        for v in range(5):
                            nc.sync.dma_start(
                                out=bnd_nt[:, v * NCCH:(v + 1) * NCCH],
                                in_=scrC[v:v + 1, :].rearrange(
                                    "o (p c) -> p (o c)", p=P))
                        st_ = bnd_nt[:, 0:NCCH]
                        rym = bnd_nt[:, NCCH:2 * NCCH]
                        # n-domain: s, satl, zz, loss
                        s_nt = sm.tile([P, NCCH], dtf, tag="s_nt")
                        zz = sm.tile([P, NCCH], dtf, tag="zz")
                        tmp = sm.tile([P, NCCH], dtf, tag="tmp")
                        nc.vector.tensor_tensor(s_nt[:], a_t[:], st_, MUL)
                        nc.vector.tensor_sub(satl[:], satl[:], s_nt[:])
                        nc.vector.tensor_scalar(out=satl[:], in0=satl[:], scalar1=0.0,
                                                scalar2=None, op0=AluOpType.max)
                        scrS = dscr.tile([1, N], dtf, tag="scrS")
                        nc.sync.dma_start(
                            out=scrS[0:1, :].rearrange("o (p c) -> p (o c)", p=P),
                            in_=satl[:])
                        nc.sync.dma_start(out=satl_row[:], in_=scrS[:])
                        nc.vector.scalar_tensor_tensor(zz[:], st_, 1.0, xn_nt[:],
                                                       MUL, MUL)
                        nc.vector.tensor_tensor(zz[:], zz[:], rym, ADD)
                        for d in range(3):
                            ryd = bnd_nt[:, (2 + d) * NCCH:(3 + d) * NCCH]
                            nc.vector.scalar_tensor_tensor(
                                tmp[:], ryd, -2.0,
                                xc_nt[:, d * NCCH:(d + 1) * NCCH], MUL, MUL)
                            nc.vector.tensor_tensor(zz[:], zz[:], tmp[:], ADD)
                        nc.vector.tensor_tensor(zz[:], zz[:], a_t[:], MUL)
                        nc.vector.tensor_tensor(lossacc[:], lossacc[:], zz[:], ADD)
                        satr_lp_prev = satr_lp
                        satrp_prev = satrp
                    # final per-batch reduction
                    lsum = sm.tile([P, 1], dtf, tag="lsum")
                    nc.vector.reduce_sum(lsum[:], lossacc[:], axis=mybir.AxisListType.X)
                    lall = sm.tile([P, 1], dtf, tag="lall")
                    nc.gpsimd.partition_all_reduce(
                        lall[:], lsum[:], channels=P, reduce_op=bass_isa.ReduceOp.add)
                    nc.sync.dma_start(out=out_d[0:1, b:b + 1], in_=lall[0:1, 0:1])
    nc.finalize()
    return nc


def _host_prep(points_x, points_y):
    px = np.ascontiguousarray(points_x, np.float32)
    py = np.ascontiguousarray(points_y, np.float32)
    in_maps = []
    for core in range(NCORES):
        xt = np.empty((BPC, D, N), np.float16)
        yt = np.empty((BPC, D, M), np.float16)
        ynt = np.empty((BPC, P, NT), np.float32)
        yc = np.empty((BPC, P, 3 * NT), np.float32)
        xadj = np.empty((BPC, 1, N), np.float16)
        rdif = np.empty((BPC, 1, N), np.float16)
        xn_nt = np.empty((BPC, P, NCCH), np.float32)
        xc_nt = np.empty((BPC, P, 3 * NCCH), np.float32)
        for b in range(BPC):
            gb = core * BPC + b
            x, y = px[gb], py[gb]
            xm2 = (np.float16(-2.0) * x.astype(np.float16))
            y16 = y.astype(np.float16)
            xt[b] = xm2.T
            yt[b] = y16.T
            xnv = (x * x).sum(-1).astype(np.float32)          # [N]
            ynv = (y * y).sum(-1).astype(np.float32)          # [M]
            ynt[b] = ynv.reshape(NT, P).T
            for d in range(D):
                yc[b, :, d * NT:(d + 1) * NT] = y[:, d].reshape(NT, P).T
            # device d2 assoc order: (fp16 cross + ym) + (xn - rowmax)
            cross = (xm2.astype(np.float32) @ y16.astype(np.float32).T).T
            d2 = cross + ynv[:, None] + xnv[None, :]          # [M, N]
            rowmax = d2.max(0)   # over m, per n
            rowmin = d2.min(0)
            xadj[b, 0] = (xnv - rowmax).astype(np.float16)
            rdif[b, 0] = (rowmax - rowmin).astype(np.float16)
            # n-domain layout: n = p * NCCH + c
            xn_nt[b] = xnv.reshape(P, NCCH)
            for d in range(D):
                xc_nt[b, :, d * NCCH:(d + 1) * NCCH] = x[:, d].reshape(P, NCCH)
        in_maps.append({"xt": xt, "yt": yt, "ynt": ynt, "yc": yc, "xadj": xadj,
                        "rdif": rdif, "xn_nt": xn_nt, "xc_nt": xc_nt})
    return in_maps


def _get_built(nsteps):
    key = (nsteps, E_BUFS, BND_CH)
    if key not in _CACHE:
        _CACHE[key] = _build(nsteps)
    return _CACHE[key]


def run(points_x, points_y, nsteps=None, trace=False):
    from concourse.bass_utils import run_bass_kernel_spmd
    nsteps = nsteps or KERNEL_NSTEPS
    nc = _get_built(nsteps)
    in_maps = _host_prep(points_x, points_y)
    res = run_bass_kernel_spmd(nc, in_maps, core_ids=list(range(NCORES)),
                               trace=trace)
    outs = np.array([res.results[c]["out"].reshape(-1) for c in range(NCORES)])
    loss = np.float32(outs.sum() / np.float32(B))
    return loss, res


def kernel(points_x, points_y):
    loss, _ = run(points_x, points_y)
    return np.asarray(loss, np.float32)


# revision 16
# speedup vs baseline: 1.1923x; 1.1923x over previous
"""Approximate EMD loss (B=16, N=M=2048, D=3) on 8 TRN2 NeuronCores. V2.

Data-parallel over batch: each core owns 2 batch items, processed
sequentially. m-layout: partition dim = m (16 tiles of 128), free dim = n.

Key redesign vs V1 (baseline 11.8ms):
- zz[n] = sum_m w*E*d2 decomposed via d2 = xn[n] + ym[m] - 2 sum_d x_d y_d,
  so the loss colsum comes out of the SAME PE pass as s (5-wide weight
  bundle: w2, w2*ym, w2*y0..2). No V/Z tiles, no GpSimd big pass, no
  DVE/GpSimd SBUF-port contention.
- ss[m] = satr * sum_n E*a[n]: DVE tensor_tensor (2x bf16) +
  tensor_scalar accum (4x) instead of scalar_tensor_tensor (1x only).
- rd colsum chunk-major (4 x 512 cols) so the rd->a->a_b dance pipelines
  per chunk and the ss pass starts ~9us into the level instead of ~22us.
- Full-width [128,16] m/n-domain ops once per level (V1: 8 groups of
  [128,2] ops -> ~60 DVE instrs/level of pure overhead + sem stalls).
- Bundle pass is m-tile-major so E buffers free progressively; E pool of
  12 x [128,4096] double-buffers ACROSS levels within 96KB/partition.
- Setup cross matmul in fp16 (1 cyc/row vs 4 for fp32).
"""
import sys
import os
import numpy as np

sys.path.insert(0, "/opt/trn_rl_repo")

B, N, M, D = 16, 2048, 2048, 3
P = 128
NT = M // P          # 16 m-tiles
NCCH = N // 128      # 16 cols in n-domain [128,16] layout
NCORES = 8
BPC = B // NCORES    # 2 batch items per core
LEVELS = np.arange(8, -3, -0.25).astype(np.float32)
LEVELS[-1] = 0.0
NSTEPS = len(LEVELS)  # 44
NEG_START = 33        # first step with negative level (d2s re-shift point)

CH = 512              # rd/ss chunk width
NCH = N // CH         # 4 chunks
E_BUFS = int(os.environ.get("EMD_E_BUFS", "12"))
BND_CH = int(os.environ.get("EMD_BND_CH", "512"))
KERNEL_NSTEPS = int(os.environ.get("EMD_NSTEPS", str(NSTEPS)))

_CACHE = {}


def _build(nsteps):
    import concourse.bacc as bacc
    import concourse.mybir as mybir
    import concourse.bass_isa as bass_isa
    from concourse.tile import TileContext
    from concourse.alu_op_type import AluOpType

    dtf = mybir.dt.float32
    dtb = mybir.dt.bfloat16
    dth = mybir.dt.float16
    AF = mybir.ActivationFunctionType
    MUL = AluOpType.mult
    ADD = AluOpType.add

    nc = bacc.Bacc(None, target_bir_lowering=False)
    xt_d = nc.dram_tensor("xt", [BPC, D, N], dth, kind="ExternalInput")     # (-2x).T
    yt_d = nc.dram_tensor("yt", [BPC, D, M], dth, kind="ExternalInput")     # y.T
    ynt_d = nc.dram_tensor("ynt", [BPC, P, NT], dtf, kind="ExternalInput")  # ym m-major
    yc_d = nc.dram_tensor("yc", [BPC, P, 3 * NT], dtf, kind="ExternalInput")   # y coords m-major
    xadj_d = nc.dram_tensor("xadj", [BPC, 1, N], dth, kind="ExternalInput")  # xn - rowmax
    rdif_d = nc.dram_tensor("rdif", [BPC, 1, N], dth, kind="ExternalInput")  # rowmax - rowmin
    xn_nt_d = nc.dram_tensor("xn_nt", [BPC, P, NCCH], dtf, kind="ExternalInput")  # xn n-major
    xc_nt_d = nc.dram_tensor("xc_nt", [BPC, P, 3 * NCCH], dtf, kind="ExternalInput")  # x n-major
    out_d = nc.dram_tensor("out", [1, BPC], dtf, kind="ExternalOutput")

    with TileContext(nc) as tc:
        with tc.tile_pool(name="d2", bufs=1) as d2p, \
             tc.tile_pool(name="state", bufs=1) as stp:
            for b in range(BPC):
                d2_sb = d2p.tile([P, NT * N], dth, tag="d2")
                satr_t = stp.tile([P, NT], dtf, tag="satr")       # m-domain
                lossacc = stp.tile([P, NCCH], dtf, tag="lossacc")
                ynt_sb = stp.tile([P, NT], dtf, tag="ynt")
                yc_sb = stp.tile([P, 3 * NT], dtf, tag="yc")
                xn_nt = stp.tile([P, NCCH], dtf, tag="xn_nt")
                xc_nt = stp.tile([P, 3 * NCCH], dtf, tag="xc_nt")
                satl_row = stp.tile([1, N], dtf, tag="satl_row")
                nc.vector.memset(satl_row[:], 1.0)
                nc.vector.memset(satr_t[:], 1.0)
                nc.vector.memset(lossacc[:], 0.0)
                nc.sync.dma_start(out=ynt_sb[:], in_=ynt_d[b])
                nc.sync.dma_start(out=yc_sb[:], in_=yc_d[b])
                nc.sync.dma_start(out=xn_nt[:], in_=xn_nt_d[b])
                nc.sync.dma_start(out=xc_nt[:], in_=xc_nt_d[b])

                # ---- setup: d2s = ((-2 y.x) + ym) + (xn - rowmax), fp16
                with tc.tile_pool(name="su", bufs=1) as su, \
                     tc.tile_pool(name="sups", bufs=4, space="PSUM") as sups, \
                     tc.tile_pool(name="sutmp", bufs=4) as sutmp:
                    xt_sb = su.tile([D, N], dth, tag="xt")
                    yt_sb = su.tile([D, M], dth, tag="yt")
                    xadj_sb = su.tile([1, N], dth, tag="xadj")
                    xadj_b = su.tile([P, N], dth, tag="xadj_b")
                    nc.sync.dma_start(out=xt_sb[:], in_=xt_d[b])
                    nc.sync.dma_start(out=yt_sb[:], in_=yt_d[b])
                    nc.sync.dma_start(out=xadj_sb[:], in_=xadj_d[b])
                    nc.gpsimd.partition_broadcast(xadj_b[:], xadj_sb[0:1, :])
                    for i in range(NT):
                        for c in range(NCH):
                            cps = sups.tile([P, CH], dtf, tag="cross")
                            nc.tensor.matmul(
                                cps[:], yt_sb[:, i * P:(i + 1) * P],
                                xt_sb[:, c * CH:(c + 1) * CH])
                            # cross+ym on ScalarE (per-partition bias), then
                            # +xadj on DVE at fp16 2x
                            ctmp = sutmp.tile([P, CH], dth, tag="ctmp")
                            nc.scalar.activation(ctmp[:], cps[:], AF.Identity,
                                                 bias=ynt_sb[:, i:i + 1])
                            nc.vector.tensor_tensor(
                                d2_sb[:, i * N + c * CH: i * N + (c + 1) * CH],
                                ctmp[:], xadj_b[:, c * CH:(c + 1) * CH], ADD)

                # ---- 44-level matching loop
                with tc.tile_pool(name="ep", bufs=E_BUFS) as ep, \
                     tc.tile_pool(name="ps", bufs=2) as psc, \
                     tc.tile_pool(name="ab", bufs=1) as abp, \
                     tc.tile_pool(name="sh", bufs=1) as shp, \
                     tc.tile_pool(name="sm", bufs=2) as sm, \
                     tc.tile_pool(name="rows", bufs=4) as rowp, \
                     tc.tile_pool(name="brows", bufs=2) as browp, \
                     tc.tile_pool(name="dscr", bufs=2, space="DRAM") as dscr, \
                     tc.tile_pool(name="rps", bufs=1, space="PSUM") as rps, \
                     tc.tile_pool(name="bps", bufs=1, space="PSUM") as bps:
                    satr_lp_prev = sm.tile([P, NT], dtb, tag="satr_lp")
                    satrp_prev = sm.tile([P, NT], dtf, tag="satrp")
                    nc.vector.memset(satr_lp_prev[:], 1.0)
                    nc.vector.memset(satrp_prev[:], 1.0)
                    for s in range(nsteps):
                        lv = float(LEVELS[s])
                        if s == NEG_START:
                            # re-shift d2s to d2 - rowmin for negative levels
                            shrow = shp.tile([1, N], dth, tag="shrow")
                            shift_b = shp.tile([P, N], dth, tag="shift_b")
                            nc.sync.dma_start(out=shrow[:], in_=rdif_d[b])
                            nc.gpsimd.partition_broadcast(shift_b[:], shrow[0:1, :])
                            for i in range(NT):
                                nc.vector.tensor_tensor(
                                    d2_sb[:, i * N:(i + 1) * N],
                                    d2_sb[:, i * N:(i + 1) * N], shift_b[:], ADD)
                        # E tiles (bf16): 8 ACTs of FD=4096, each covering 2
                        # m-tiles. E_ap(i,...) = AP for m-tile i's columns.
                        E_bufs = []
                        for j in range(NT // 2):
                            E = ep.tile([P, 2 * N], dtb, tag="E")
                            nc.scalar.activation(E[:], d2_sb[:, j * 2 * N:(j + 1) * 2 * N],
                                                 AF.Exp, scale=lv)
                            E_bufs.append(E)

                        def E_ap(i, c0, c1):
                            return E_bufs[i // 2][:, (i % 2) * N + c0:(i % 2) * N + c1]

                        # rd pass, chunk-major + pipelined a-dance + ss chunks
                        rd_ps = rps.tile([1, N], dtf, tag="rd_ps")
                        a_t = sm.tile([P, NCCH], dtb, tag="a_t")
                        a_b = abp.tile([P, N], dtb, tag="a_b")
                        a_row = abp.tile([1, N], dtb, tag="a_row")
                        scrB = dscr.tile([1, N], dtb, tag="scrB")
                        ssacc = sm.tile([P, 2 * NT], dtf, tag="ssacc")
                        sdum = psc.tile([P, 1], dtb, tag="sdum")
                        for c in range(NCH):
                            lo, hi = c * CH, (c + 1) * CH
                            for i in range(NT):
                                nc.tensor.matmul(
                                    rd_ps[0:1, lo:hi],
                                    satr_lp_prev[:, i:i + 1],
                                    E_ap(i, lo, hi),
                                    start=(i == 0), stop=(i == NT - 1))
                            # a-route chunk c, all in row form (no DMA on the
                            # critical path): a = satl/rd
                            rrowi = rowp.tile([1, CH], dtf, tag="rrowi")
                            nc.vector.reciprocal_approx_fast(
                                out=rrowi[:], in_=rd_ps[0:1, lo:hi])
                            nc.vector.tensor_tensor(a_row[0:1, lo:hi], rrowi[:],
                                                    satl_row[0:1, lo:hi], MUL)
                            nc.gpsimd.partition_broadcast(a_b[:, lo:hi],
                                                          a_row[0:1, lo:hi])
                            # off-critical: a to n-domain for the level tail
                            nc.sync.dma_start(out=scrB[0:1, lo:hi],
                                              in_=a_row[0:1, lo:hi])
                            pl, ph = 32 * c, 32 * c + 32
                            nc.sync.dma_start(
                                out=a_t[pl:ph, :],
                                in_=scrB[0:1, lo:hi].rearrange(
                                    "o (p c) -> p (o c)", p=32))
                            # ss partials per half: one fused 1x DVE op per
                            # tile: accum = sum_n (E*satr)*a_b (no writes)
                            if c % 2 == 1:
                                h = c // 2
                                hlo, hhi = h * 2 * CH, (h + 1) * 2 * CH
                                for i in range(NT):
                                    nc.vector.affine_mul_reduce(
                                        out=sdum.broadcast_to((P, 2 * CH)),
                                        accum_out=ssacc[:, h * NT + i:h * NT + i + 1],
                                        in0=E_ap(i, hlo, hhi),
                                        in1=a_b[:, hlo:hhi],
                                        scale=satrp_prev[:, i:i + 1],
                                        bias=0.0)
                                    if i % 3 == 2:
                                        # HAM keep-warm: trivial matmul chained
                                        # to this ss op so PE stays at 2.4GHz
                                        # through the DVE window
                                        nc.tensor.matmul(
                                            rd_ps[0:1, 0:1],
                                            satr_lp_prev[:, 0:1],
                                            sdum[:])
                        # ss (satr already folded via scale)
                        ss = sm.tile([P, NT], dtf, tag="ss")
                        nc.vector.tensor_tensor(ss[:], ssacc[:, 0:NT],
                                                ssacc[:, NT:2 * NT], ADD)
                        # m-domain: sc2, satr update, bundle weights
                        ssp = sm.tile([P, NT], dtf, tag="ssp")
                        sinv = sm.tile([P, NT], dtf, tag="sinv")
                        sc2 = sm.tile([P, NT], dtf, tag="sc2")
                        mn = sm.tile([P, NT], dtf, tag="mn")
                        satrp = sm.tile([P, NT], dtf, tag="satrp")
                        satr_lp = sm.tile([P, NT], dtb, tag="satr_lp")
                        w2 = sm.tile([P, NT], dtf, tag="w2")
                        w2all = sm.tile([P, NT * 5], dtb, tag="w2all")
                        nc.vector.tensor_scalar(out=ssp[:], in0=ss[:], scalar1=1e-9,
                                                scalar2=None, op0=ADD)
                        nc.vector.reciprocal(sinv[:], ssp[:])
                        nc.vector.tensor_tensor(sc2[:], sinv[:], satr_t[:], MUL)
                        nc.vector.tensor_scalar(out=sc2[:], in0=sc2[:], scalar1=1.0,
                                                scalar2=None, op0=AluOpType.min)
                        nc.vector.tensor_tensor(w2[:], sc2[:], satrp_prev[:], MUL)
                        # bundle lhsT, interleaved [P, NT, 5]
                        w2v = w2all[:].rearrange("p (t f) -> p t f", f=5)
                        nc.vector.tensor_copy(w2v[:, :, 0], w2[:])
                        nc.vector.tensor_tensor(w2v[:, :, 1], w2[:], ynt_sb[:], MUL)
                        nc.vector.tensor_tensor(w2v[:, :, 2], w2[:], yc_sb[:, 0:NT], MUL)
                        nc.vector.tensor_tensor(w2v[:, :, 3], w2[:], yc_sb[:, NT:2 * NT], MUL)
                        nc.vector.tensor_tensor(w2v[:, :, 4], w2[:], yc_sb[:, 2 * NT:3 * NT], MUL)
                        # satr' = satr - min(ss, satr)  (>= 0 by construction)
                        nc.vector.tensor_tensor(mn[:], ss[:], satr_t[:], AluOpType.min)
                        nc.vector.tensor_sub(satr_t[:], satr_t[:], mn[:])
                        nc.vector.tensor_scalar(out=satrp[:], in0=satr_t[:],
                                                scalar1=1e-30, scalar2=None, op0=ADD)
                        nc.vector.tensor_copy(satr_lp[:], satrp[:])
                        # bundle pass: 5-wide colsums of E, m-tile-major so E
                        # buffers free progressively for next level's ACTs
                        bnd_ps = bps.tile([5, N], dtf, tag="bnd_ps")
                        for i in range(NT):
                            for c in range(N // BND_CH):
                                lo, hi = c * BND_CH, (c + 1) * BND_CH
                                nc.tensor.matmul(
                                    bnd_ps[0:5, lo:hi],
                                    w2all[:, i * 5:(i + 1) * 5],
                                    E_ap(i, lo, hi),
                                    start=(i == 0), stop=(i == NT - 1))
                        # satl' = relu(satl - a*s~), all in row form straight
                        # from bundle PSUM (keeps the next level's a-route off
                        # the DMA queues)
                        tmp_row = abp.tile([1, N], dtf, tag="tmp_row")
                        nc.vector.scalar_tensor_tensor(
                            tmp_row[:], bnd_ps[0:1, :], -1.0, a_row[0:1, :],
                            MUL, MUL)
                        nc.vector.tensor_tensor(satl_row[:], satl_row[:],
                                                tmp_row[:], ADD)
                        nc.vector.tensor_scalar(out=satl_row[:], in0=satl_row[:],
                                                scalar1=0.0, scalar2=None,
                                                op0=AluOpType.max)
                        # end dance: bundle rows -> n-domain (chunked copies)
                        scrC = dscr.tile([5, N], dtf, tag="scrC")
                        for c in range(NCH):
                            lo, hi = c * CH, (c + 1) * CH
                            brow = browp.tile([5, CH], dtf, tag="brow")
                            nc.scalar.copy(brow[:], bnd_ps[0:5, lo:hi])
                            nc.sync.dma_start(out=scrC[0:5, lo:hi], in_=brow[:])
                        bnd_nt = sm.tile([P, 5 * NCCH], dtf, tag="bnd_nt")
                        for v in range(5):
                            nc.sync.dma_start(
                                out=bnd_nt[:, v * NCCH:(v + 1) * NCCH],
                                in_=scrC[v:v + 1, :].rearrange(
                                    "o (p c) -> p (o c)", p=P))
                        st_ = bnd_nt[:, 0:NCCH]
                        rym = bnd_nt[:, NCCH:2 * NCCH]
                        # n-domain: zz, loss
                        zz = sm.tile([P, NCCH], dtf, tag="zz")
                        tmp = sm.tile([P, NCCH], dtf, tag="tmp")
                        nc.vector.scalar_tensor_tensor(zz[:], st_, 1.0, xn_nt[:],
                                                       MUL, MUL)
                        nc.vector.tensor_tensor(zz[:], zz[:], rym, ADD)
                        for d in range(3):
                            ryd = bnd_nt[:, (2 + d) * NCCH:(3 + d) * NCCH]
                            nc.vector.scalar_tensor_tensor(
                                tmp[:], ryd, -2.0,
                                xc_nt[:, d * NCCH:(d + 1) * NCCH], MUL, MUL)
                            nc.vector.tensor_tensor(zz[:], zz[:], tmp[:], ADD)
                        nc.vector.tensor_tensor(zz[:], zz[:], a_t[:], MUL)
                        nc.vector.tensor_tensor(lossacc[:], lossacc[:], zz[:], ADD)
                        satr_lp_prev = satr_lp
                        satrp_prev = satrp
                    # final per-batch reduction
                    lsum = sm.tile([P, 1], dtf, tag="lsum")
                    nc.vector.reduce_sum(lsum[:], lossacc[:], axis=mybir.AxisListType.X)
                    lall = sm.tile([P, 1], dtf, tag="lall")
                    nc.gpsimd.partition_all_reduce(
                        lall[:], lsum[:], channels=P, reduce_op=bass_isa.ReduceOp.add)
                    nc.sync.dma_start(out=out_d[0:1, b:b + 1], in_=lall[0:1, 0:1])
    nc.finalize()
    return nc


def _host_prep(points_x, points_y):
    px = np.ascontiguousarray(points_x, np.float32)
    py = np.ascontiguousarray(points_y, np.float32)
    in_maps = []
    for core in range(NCORES):
        xt = np.empty((BPC, D, N), np.float16)
        yt = np.empty((BPC, D, M), np.float16)
        ynt = np.empty((BPC, P, NT), np.float32)
        yc = np.empty((BPC, P, 3 * NT), np.float32)
        xadj = np.empty((BPC, 1, N), np.float16)
        rdif = np.empty((BPC, 1, N), np.float16)
        xn_nt = np.empty((BPC, P, NCCH), np.float32)
        xc_nt = np.empty((BPC, P, 3 * NCCH), np.float32)
        for b in range(BPC):
            gb = core * BPC + b
            x, y = px[gb], py[gb]
            xm2 = (np.float16(-2.0) * x.astype(np.float16))
            y16 = y.astype(np.float16)
            xt[b] = xm2.T
            yt[b] = y16.T
            xnv = (x * x).sum(-1).astype(np.float32)          # [N]
            ynv = (y * y).sum(-1).astype(np.float32)          # [M]
            ynt[b] = ynv.reshape(NT, P).T
            for d in range(D):
                yc[b, :, d * NT:(d + 1) * NT] = y[:, d].reshape(NT, P).T
            # device d2 assoc order: (fp16 cross + ym) + (xn - rowmax)
            cross = (xm2.astype(np.float32) @ y16.astype(np.float32).T).T
            d2 = cross + ynv[:, None] + xnv[None, :]          # [M, N]
            rowmax = d2.max(0)   # over m, per n
            rowmin = d2.min(0)
            xadj[b, 0] = (xnv - rowmax).astype(np.float16)
            rdif[b, 0] = (rowmax - rowmin).astype(np.float16)
            # n-domain layout: n = p * NCCH + c
            xn_nt[b] = xnv.reshape(P, NCCH)
            for d in range(D):
                xc_nt[b, :, d * NCCH:(d + 1) * NCCH] = x[:, d].reshape(P, NCCH)
        in_maps.append({"xt": xt, "yt": yt, "ynt": ynt, "yc": yc, "xadj": xadj,
                        "rdif": rdif, "xn_nt": xn_nt, "xc_nt": xc_nt})
    return in_maps


def _get_built(nsteps):
    key = (nsteps, E_BUFS, BND_CH)
    if key not in _CACHE:
        _CACHE[key] = _build(nsteps)
    return _CACHE[key]


def run(points_x, points_y, nsteps=None, trace=False):
    from concourse.bass_utils import run_bass_kernel_spmd
    nsteps = nsteps or KERNEL_NSTEPS
    nc = _get_built(nsteps)
    in_maps = _host_prep(points_x, points_y)
    res = run_bass_kernel_spmd(nc, in_maps, core_ids=list(range(NCORES)),
                               trace=trace)
    outs = np.array([res.results[c]["out"].reshape(-1) for c in range(NCORES)])
    loss = np.float32(outs.sum() / np.float32(B))
    return loss, res


def kernel(points_x, points_y):
    loss, _ = run(points_x, points_y)
    return np.asarray(loss, np.float32)


# revision 18
# speedup vs baseline: 1.2802x; 1.0737x over previous
"""Approximate EMD loss (B=16, N=M=2048, D=3) on 8 TRN2 NeuronCores. V2.

Data-parallel over batch: each core owns 2 batch items, processed
sequentially. m-layout: partition dim = m (16 tiles of 128), free dim = n.

Key redesign vs V1 (baseline 11.8ms):
- zz[n] = sum_m w*E*d2 decomposed via d2 = xn[n] + ym[m] - 2 sum_d x_d y_d,
  so the loss colsum comes out of the SAME PE pass as s (5-wide weight
  bundle: w2, w2*ym, w2*y0..2). No V/Z tiles, no GpSimd big pass, no
  DVE/GpSimd SBUF-port contention.
- ss[m] = satr * sum_n E*a[n]: DVE tensor_tensor (2x bf16) +
  tensor_scalar accum (4x) instead of scalar_tensor_tensor (1x only).
- rd colsum chunk-major (4 x 512 cols) so the rd->a->a_b dance pipelines
  per chunk and the ss pass starts ~9us into the level instead of ~22us.
- Full-width [128,16] m/n-domain ops once per level (V1: 8 groups of
  [128,2] ops -> ~60 DVE instrs/level of pure overhead + sem stalls).
- Bundle pass is m-tile-major so E buffers free progressively; E pool of
  12 x [128,4096] double-buffers ACROSS levels within 96KB/partition.
- Setup cross matmul in fp16 (1 cyc/row vs 4 for fp32).
"""
import sys
import os
import numpy as np

sys.path.insert(0, "/opt/trn_rl_repo")

B, N, M, D = 16, 2048, 2048, 3
P = 128
NT = M // P          # 16 m-tiles
NCCH = N // 128      # 16 cols in n-domain [128,16] layout
NCORES = 8
BPC = B // NCORES    # 2 batch items per core
LEVELS = np.arange(8, -3, -0.25).astype(np.float32)
LEVELS[-1] = 0.0
NSTEPS = len(LEVELS)  # 44
NEG_START = 33        # first step with negative level (d2s re-shift point)

CH = 512              # rd/ss chunk width
NCH = N // CH         # 4 chunks
E_BUFS = int(os.environ.get("EMD_E_BUFS", "11"))
BND_CH = int(os.environ.get("EMD_BND_CH", "512"))
KERNEL_NSTEPS = int(os.environ.get("EMD_NSTEPS", str(NSTEPS)))

_CACHE = {}


def _build(nsteps):
    import concourse.bacc as bacc
    import concourse.mybir as mybir
    import concourse.bass_isa as bass_isa
    from concourse.tile import TileContext
    from concourse.alu_op_type import AluOpType

    dtf = mybir.dt.float32
    dtb = mybir.dt.bfloat16
    dth = mybir.dt.float16
    AF = mybir.ActivationFunctionType
    MUL = AluOpType.mult
    ADD = AluOpType.add

    nc = bacc.Bacc(None, target_bir_lowering=False)
    xt_d = nc.dram_tensor("xt", [BPC, D, N], dth, kind="ExternalInput")     # (-2x).T
    yt_d = nc.dram_tensor("yt", [BPC, D, M], dth, kind="ExternalInput")     # y.T
    ynt_d = nc.dram_tensor("ynt", [BPC, P, NT], dtf, kind="ExternalInput")  # ym m-major
    yc_d = nc.dram_tensor("yc", [BPC, P, 3 * NT], dtf, kind="ExternalInput")   # y coords m-major
    xadj_d = nc.dram_tensor("xadj", [BPC, 1, N], dth, kind="ExternalInput")  # xn - rowmax
    rdif_d = nc.dram_tensor("rdif", [BPC, 1, N], dth, kind="ExternalInput")  # rowmax - rowmin
    xn_nt_d = nc.dram_tensor("xn_nt", [BPC, P, NCCH], dtf, kind="ExternalInput")  # xn n-major
    xc_nt_d = nc.dram_tensor("xc_nt", [BPC, P, 3 * NCCH], dtf, kind="ExternalInput")  # x n-major
    out_d = nc.dram_tensor("out", [1, BPC], dtf, kind="ExternalOutput")

    with TileContext(nc) as tc:
        with tc.tile_pool(name="d2", bufs=1) as d2p, \
             tc.tile_pool(name="state", bufs=1) as stp:
            for b in range(BPC):
                d2_sb = d2p.tile([P, NT * N], dth, tag="d2")
                satr_t = stp.tile([P, NT], dtf, tag="satr")       # m-domain
                lossacc = stp.tile([P, NCCH], dtf, tag="lossacc")
                ynt_sb = stp.tile([P, NT], dtf, tag="ynt")
                yc_sb = stp.tile([P, 3 * NT], dtf, tag="yc")
                xn_nt = stp.tile([P, NCCH], dtf, tag="xn_nt")
                xc_nt = stp.tile([P, 3 * NCCH], dtf, tag="xc_nt")
                satl_row = stp.tile([1, N], dtf, tag="satl_row")
                nc.vector.memset(satl_row[:], 1.0)
                nc.vector.memset(satr_t[:], 1.0)
                nc.vector.memset(lossacc[:], 0.0)
                nc.sync.dma_start(out=ynt_sb[:], in_=ynt_d[b])
                nc.sync.dma_start(out=yc_sb[:], in_=yc_d[b])
                nc.sync.dma_start(out=xn_nt[:], in_=xn_nt_d[b])
                nc.sync.dma_start(out=xc_nt[:], in_=xc_nt_d[b])

                # ---- setup: d2s = ((-2 y.x) + ym) + (xn - rowmax), fp16
                with tc.tile_pool(name="su", bufs=1) as su, \
                     tc.tile_pool(name="sups", bufs=4, space="PSUM") as sups, \
                     tc.tile_pool(name="sutmp", bufs=4) as sutmp:
                    xt_sb = su.tile([D, N], dth, tag="xt")
                    yt_sb = su.tile([D, M], dth, tag="yt")
                    xadj_sb = su.tile([1, N], dth, tag="xadj")
                    xadj_b = su.tile([P, N], dth, tag="xadj_b")
                    nc.sync.dma_start(out=xt_sb[:], in_=xt_d[b])
                    nc.sync.dma_start(out=yt_sb[:], in_=yt_d[b])
                    nc.sync.dma_start(out=xadj_sb[:], in_=xadj_d[b])
                    nc.gpsimd.partition_broadcast(xadj_b[:], xadj_sb[0:1, :])
                    for i in range(NT):
                        for c in range(NCH):
                            cps = sups.tile([P, CH], dtf, tag="cross")
                            nc.tensor.matmul(
                                cps[:], yt_sb[:, i * P:(i + 1) * P],
                                xt_sb[:, c * CH:(c + 1) * CH])
                            # cross+ym on ScalarE (per-partition bias), then
                            # +xadj on DVE at fp16 2x
                            ctmp = sutmp.tile([P, CH], dth, tag="ctmp")
                            nc.scalar.activation(ctmp[:], cps[:], AF.Identity,
                                                 bias=ynt_sb[:, i:i + 1])
                            nc.vector.tensor_tensor(
                                d2_sb[:, i * N + c * CH: i * N + (c + 1) * CH],
                                ctmp[:], xadj_b[:, c * CH:(c + 1) * CH], ADD)

                # ---- 44-level matching loop
                with tc.tile_pool(name="ep", bufs=E_BUFS) as ep, \
                     tc.tile_pool(name="ps", bufs=2) as psc, \
                     tc.tile_pool(name="ab", bufs=1) as abp, \
                     tc.tile_pool(name="sh", bufs=1) as shp, \
                     tc.tile_pool(name="sm", bufs=2) as sm, \
                     tc.tile_pool(name="rows", bufs=4) as rowp, \
                     tc.tile_pool(name="brows", bufs=2) as browp, \
                     tc.tile_pool(name="dscr", bufs=2, space="DRAM") as dscr, \
                     tc.tile_pool(name="rps", bufs=1, space="PSUM") as rps, \
                     tc.tile_pool(name="bps", bufs=1, space="PSUM") as bps:
                    satr_lp_prev = sm.tile([P, NT], dtb, tag="satr_lp")
                    satrp_prev = sm.tile([P, NT], dtf, tag="satrp")
                    nc.vector.memset(satr_lp_prev[:], 1.0)
                    nc.vector.memset(satrp_prev[:], 1.0)
                    for s in range(nsteps):
                        lv = float(LEVELS[s])
                        if s == NEG_START:
                            # re-shift d2s to d2 - rowmin for negative levels
                            shrow = shp.tile([1, N], dth, tag="shrow")
                            shift_b = shp.tile([P, N], dth, tag="shift_b")
                            nc.sync.dma_start(out=shrow[:], in_=rdif_d[b])
                            nc.gpsimd.partition_broadcast(shift_b[:], shrow[0:1, :])
                            for i in range(NT):
                                nc.vector.tensor_tensor(
                                    d2_sb[:, i * N:(i + 1) * N],
                                    d2_sb[:, i * N:(i + 1) * N], shift_b[:], ADD)
                        # E tiles (bf16): 8 ACTs of FD=4096, each covering 2
                        # m-tiles. E_ap(i,...) = AP for m-tile i's columns.
                        E_bufs = []
                        for j in range(NT // 2):
                            E = ep.tile([P, 2 * N], dtb, tag="E")
                            nc.scalar.activation(E[:], d2_sb[:, j * 2 * N:(j + 1) * 2 * N],
                                                 AF.Exp, scale=lv)
                            E_bufs.append(E)

                        def E_ap(i, c0, c1):
                            return E_bufs[i // 2][:, (i % 2) * N + c0:(i % 2) * N + c1]

                        # rd pass, chunk-major + pipelined a-dance + ss chunks
                        rd_ps = rps.tile([1, N], dtf, tag="rd_ps")
                        a_t = sm.tile([P, NCCH], dtb, tag="a_t")
                        a_b = abp.tile([P, N], dtb, tag="a_b")
                        a_row = abp.tile([1, N], dtb, tag="a_row")
                        scrB = dscr.tile([1, N], dtb, tag="scrB")
                        ssacc = sm.tile([P, 2 * NT], dtf, tag="ssacc")
                        ss = sm.tile([P, NT], dtf, tag="ss")
                        sdum = psc.tile([P, 1], dtb, tag="sdum")
                        for c in range(NCH):
                            lo, hi = c * CH, (c + 1) * CH
                            for i in range(NT):
                                nc.tensor.matmul(
                                    rd_ps[0:1, lo:hi],
                                    satr_lp_prev[:, i:i + 1],
                                    E_ap(i, lo, hi),
                                    start=(i == 0), stop=(i == NT - 1))
                            # a-route chunk c, all in row form (no DMA on the
                            # critical path): a = satl/rd
                            rrowi = rowp.tile([1, CH], dtf, tag="rrowi")
                            nc.vector.reciprocal_approx_fast(
                                out=rrowi[:], in_=rd_ps[0:1, lo:hi])
                            nc.vector.tensor_tensor(a_row[0:1, lo:hi], rrowi[:],
                                                    satl_row[0:1, lo:hi], MUL)
                            nc.gpsimd.partition_broadcast(a_b[:, lo:hi],
                                                          a_row[0:1, lo:hi])
                            # off-critical: a to n-domain for the level tail
                            nc.sync.dma_start(out=scrB[0:1, lo:hi],
                                              in_=a_row[0:1, lo:hi])
                            pl, ph = 32 * c, 32 * c + 32
                            nc.sync.dma_start(
                                out=a_t[pl:ph, :],
                                in_=scrB[0:1, lo:hi].rearrange(
                                    "o (p c) -> p (o c)", p=32))
                            # ss partials per half: one fused 1x DVE op per
                            # tile: accum = sum_n (E*satr)*a_b (no writes)
                            if c % 2 == 1:
                                h = c // 2
                                hlo, hhi = h * 2 * CH, (h + 1) * 2 * CH
                                for i in range(12):
                                    nc.vector.affine_mul_reduce(
                                        out=sdum.broadcast_to((P, 2 * CH)),
                                        accum_out=ssacc[:, h * NT + i:h * NT + i + 1],
                                        in0=E_ap(i, hlo, hhi),
                                        in1=a_b[:, hlo:hhi],
                                        scale=satrp_prev[:, i:i + 1],
                                        bias=0.0)
                                    if i % 3 == 2:
                                        # HAM keep-warm: trivial matmul chained
                                        # to this ss op so PE stays at 2.4GHz
                                        # through the DVE window
                                        nc.tensor.matmul(
                                            rd_ps[0:1, 0:1],
                                            satr_lp_prev[:, 0:1],
                                            sdum[:])
                        # offloaded ss tiles 12..15: DVE multiply at 2x,
                        # reduce on ScalarE (ACT Identity + accum, satr via
                        # per-partition scale) straight into ss columns
                        for i in range(12, NT):
                            pfull = psc.tile([P, N], dtb, tag="pfull")
                            nc.vector.tensor_tensor(pfull[:], E_ap(i, 0, N),
                                                    a_b[:], MUL)
                            nc.scalar.activation(pfull[:], pfull[:], AF.Identity,
                                                 scale=satrp_prev[:, i:i + 1],
                                                 accum_out=ss[:, i:i + 1])
                        # ss (satr already folded via scale)
                        nc.vector.tensor_tensor(ss[:, 0:12], ssacc[:, 0:12],
                                                ssacc[:, NT:NT + 12], ADD)
                        # m-domain: sc2, satr update, bundle weights
                        ssp = sm.tile([P, NT], dtf, tag="ssp")
                        sinv = sm.tile([P, NT], dtf, tag="sinv")
                        sc2 = sm.tile([P, NT], dtf, tag="sc2")
                        mn = sm.tile([P, NT], dtf, tag="mn")
                        satrp = sm.tile([P, NT], dtf, tag="satrp")
                        satr_lp = sm.tile([P, NT], dtb, tag="satr_lp")
                        w2 = sm.tile([P, NT], dtf, tag="w2")
                        w2all = sm.tile([P, NT * 5], dtb, tag="w2all")
                        nc.vector.tensor_scalar(out=ssp[:], in0=ss[:], scalar1=1e-9,
                                                scalar2=None, op0=ADD)
                        nc.vector.reciprocal(sinv[:], ssp[:])
                        nc.vector.tensor_tensor(sc2[:], sinv[:], satr_t[:], MUL)
                        nc.vector.tensor_scalar(out=sc2[:], in0=sc2[:], scalar1=1.0,
                                                scalar2=None, op0=AluOpType.min)
                        nc.vector.tensor_tensor(w2[:], sc2[:], satrp_prev[:], MUL)
                        # bundle lhsT, interleaved [P, NT, 5]
                        w2v = w2all[:].rearrange("p (t f) -> p t f", f=5)
                        nc.vector.tensor_copy(w2v[:, :, 0], w2[:])
                        nc.vector.tensor_tensor(w2v[:, :, 1], w2[:], ynt_sb[:], MUL)
                        nc.vector.tensor_tensor(w2v[:, :, 2], w2[:], yc_sb[:, 0:NT], MUL)
                        nc.vector.tensor_tensor(w2v[:, :, 3], w2[:], yc_sb[:, NT:2 * NT], MUL)
                        nc.vector.tensor_tensor(w2v[:, :, 4], w2[:], yc_sb[:, 2 * NT:3 * NT], MUL)
                        # satr' = satr - min(ss, satr)  (>= 0 by construction)
                        nc.vector.tensor_tensor(mn[:], ss[:], satr_t[:], AluOpType.min)
                        nc.vector.tensor_sub(satr_t[:], satr_t[:], mn[:])
                        nc.vector.tensor_scalar(out=satrp[:], in0=satr_t[:],
                                                scalar1=1e-30, scalar2=None, op0=ADD)
                        nc.vector.tensor_copy(satr_lp[:], satrp[:])
                        # bundle pass: 5-wide colsums of E, m-tile-major so E
                        # buffers free progressively for next level's ACTs
                        bnd_ps = bps.tile([5, N], dtf, tag="bnd_ps")
                        for i in range(NT):
                            for c in range(N // BND_CH):
                                lo, hi = c * BND_CH, (c + 1) * BND_CH
                                nc.tensor.matmul(
                                    bnd_ps[0:5, lo:hi],
                                    w2all[:, i * 5:(i + 1) * 5],
                                    E_ap(i, lo, hi),
                                    start=(i == 0), stop=(i == NT - 1))
                        # satl' = relu(satl - a*s~), all in row form straight
                        # from bundle PSUM (keeps the next level's a-route off
                        # the DMA queues)
                        tmp_row = abp.tile([1, N], dtf, tag="tmp_row")
                        nc.vector.scalar_tensor_tensor(
                            tmp_row[:], bnd_ps[0:1, :], -1.0, a_row[0:1, :],
                            MUL, MUL)
                        nc.vector.tensor_tensor(satl_row[:], satl_row[:],
                                                tmp_row[:], ADD)
                        nc.vector.tensor_scalar(out=satl_row[:], in0=satl_row[:],
                                                scalar1=0.0, scalar2=None,
                                                op0=AluOpType.max)
                        # end dance: bundle rows -> n-domain (chunked copies)
                        scrC = dscr.tile([5, N], dtf, tag="scrC")
                        for c in range(NCH):
                            lo, hi = c * CH, (c + 1) * CH
                            brow = browp.tile([5, CH], dtf, tag="brow")
                            nc.scalar.copy(brow[:], bnd_ps[0:5, lo:hi])
                            nc.sync.dma_start(out=scrC[0:5, lo:hi], in_=brow[:])
                        bnd_nt = sm.tile([P, 5 * NCCH], dtf, tag="bnd_nt")
                        for v in range(5):
                            nc.sync.dma_start(
                                out=bnd_nt[:, v * NCCH:(v + 1) * NCCH],
                                in_=scrC[v:v + 1, :].rearrange(
                                    "o (p c) -> p (o c)", p=P))
                        st_ = bnd_nt[:, 0:NCCH]
                        rym = bnd_nt[:, NCCH:2 * NCCH]
                        # n-domain: zz, loss
                        zz = sm.tile([P, NCCH], dtf, tag="zz")
                        tmp = sm.tile([P, NCCH], dtf, tag="tmp")
                        nc.vector.scalar_tensor_tensor(zz[:], st_, 1.0, xn_nt[:],
                                                       MUL, MUL)
                        nc.vector.tensor_tensor(zz[:], zz[:], rym, ADD)
                        for d in range(3):
                            ryd = bnd_nt[:, (2 + d) * NCCH:(3 + d) * NCCH]
                            nc.vector.scalar_tensor_tensor(
                                tmp[:], ryd, -2.0,
                                xc_nt[:, d * NCCH:(d + 1) * NCCH], MUL, MUL)
                            nc.vector.tensor_tensor(zz[:], zz[:], tmp[:], ADD)
                        nc.vector.tensor_tensor(zz[:], zz[:], a_t[:], MUL)
                        nc.vector.tensor_tensor(lossacc[:], lossacc[:], zz[:], ADD)
                        satr_lp_prev = satr_lp
                        satrp_prev = satrp
                    # final per-batch reduction
                    lsum = sm.tile([P, 1], dtf, tag="lsum")
                    nc.vector.reduce_sum(lsum[:], lossacc[:], axis=mybir.AxisListType.X)
                    lall = sm.tile([P, 1], dtf, tag="lall")
                    nc.gpsimd.partition_all_reduce(
                        lall[:], lsum[:], channels=P, reduce_op=bass_isa.ReduceOp.add)
                    nc.sync.dma_start(out=out_d[0:1, b:b + 1], in_=lall[0:1, 0:1])
    nc.finalize()
    return nc


def _host_prep(points_x, points_y):
    px = np.ascontiguousarray(points_x, np.float32)
    py = np.ascontiguousarray(points_y, np.float32)
    in_maps = []
    for core in range(NCORES):
        xt = np.empty((BPC, D, N), np.float16)
        yt = np.empty((BPC, D, M), np.float16)
        ynt = np.empty((BPC, P, NT), np.float32)
        yc = np.empty((BPC, P, 3 * NT), np.float32)
        xadj = np.empty((BPC, 1, N), np.float16)
        rdif = np.empty((BPC, 1, N), np.float16)
        xn_nt = np.empty((BPC, P, NCCH), np.float32)
        xc_nt = np.empty((BPC, P, 3 * NCCH), np.float32)
        for b in range(BPC):
            gb = core * BPC + b
            x, y = px[gb], py[gb]
            xm2 = (np.float16(-2.0) * x.astype(np.float16))
            y16 = y.astype(np.float16)
            xt[b] = xm2.T
            yt[b] = y16.T
            xnv = (x * x).sum(-1).astype(np.float32)          # [N]
            ynv = (y * y).sum(-1).astype(np.float32)          # [M]
            ynt[b] = ynv.reshape(NT, P).T
            for d in range(D):
                yc[b, :, d * NT:(d + 1) * NT] = y[:, d].reshape(NT, P).T
            # device d2 assoc order: (fp16 cross + ym) + (xn - rowmax)
            cross = (xm2.astype(np.float32) @ y16.astype(np.float32).T).T
            d2 = cross + ynv[:, None] + xnv[None, :]          # [M, N]
            rowmax = d2.max(0)   # over m, per n
            rowmin = d2.min(0)
            xadj[b, 0] = (xnv - rowmax).astype(np.float16)
            rdif[b, 0] = (rowmax - rowmin).astype(np.float16)
            # n-domain layout: n = p * NCCH + c
            xn_nt[b] = xnv.reshape(P, NCCH)
            for d in range(D):
                xc_nt[b, :, d * NCCH:(d + 1) * NCCH] = x[:, d].reshape(P, NCCH)
        in_maps.append({"xt": xt, "yt": yt, "ynt": ynt, "yc": yc, "xadj": xadj,
                        "rdif": rdif, "xn_nt": xn_nt, "xc_nt": xc_nt})
    return in_maps


def _get_built(nsteps):
    key = (nsteps, E_BUFS, BND_CH)
    if key not in _CACHE:
        _CACHE[key] = _build(nsteps)
    return _CACHE[key]


def run(points_x, points_y, nsteps=None, trace=False):
    from concourse.bass_utils import run_bass_kernel_spmd
    nsteps = nsteps or KERNEL_NSTEPS
    nc = _get_built(nsteps)
    in_maps = _host_prep(points_x, points_y)
    res = run_bass_kernel_spmd(nc, in_maps, core_ids=list(range(NCORES)),
                               trace=trace)
    outs = np.array([res.results[c]["out"].reshape(-1) for c in range(NCORES)])
    loss = np.float32(outs.sum() / np.float32(B))
    return loss, res


def kernel(points_x, points_y):
    loss, _ = run(points_x, points_y)
    return np.asarray(loss, np.float32)
